# revision 36
# baseline (speedup 1.0000x reference)
"""Trainium2 Bass kernel for nn_AttentionCroiseeVariables.

Reference computation (N=4 vars, B=4, T=512, D=512, H=8, DK=DV=64):
  q,k,v = per-var projections of x; all-pairs (q_var, k_var) attention with
  per-key-var softmax; per-pair output projection; mean over key vars;
  residual + LayerNorm.

Sharding: 8 cores = (B=4) x (T split in 2 halves of 256 query tokens).
Core ci handles b = ci // 2, query-token half th = ci % 2.  Each core
computes its queries' attention over ALL key/value vars at full T=512.

v3 design: the kernel is ACT(exp)-bound (~134us of exp), so everything
is organized to keep the exp stream gapless:
  - score PSUM tiles form a 3-deep ring (6 banks), so the scores feeding
    exp k only wait on exp k-6 -- one extra exp period of slack vs the
    2-deep ring, enough to hide the score matmuls entirely.
  - den/av matmuls for block b are emitted AFTER the scores of block
    b+1, taking them off the exp-to-exp critical chain.
  - proj/outproj accumulators share the den/av PSUM banks as transient
    ring tiles (alternating), freeing the 2 banks a dedicated pool used.
  - 16 exp units run on DVE via a bf16-bit Schraudolph trick.
  - LayerNorm normalize is deferred and gated so the single exp->sqrt
    ACT table switch happens once, after the last attention exp.
"""

import sys
from collections import deque

import numpy as np

try:
    import concourse.bass as bass  # noqa: F401
except Exception:  # pragma: no cover
    sys.path.insert(0, "/opt/trn_rl_repo")

import ml_dtypes

import concourse.bass as bass
import concourse.tile as tile
from concourse import bacc, mybir
from concourse.bass_utils import run_bass_kernel_spmd

BF = mybir.dt.bfloat16
F32 = mybir.dt.float32
I16 = mybir.dt.int16
AF = mybir.ActivationFunctionType
OP = mybir.AluOpType

N, B, T, D = 4, 4, 512, 512
H, DK, DV = 8, 64, 64
TH = T // 2          # query tokens per core
NTOK = N * T         # kv tokens per core (all vars, one batch)
LN_EPS = 1e-5
SCALE = 1.0 / np.sqrt(DK)

# Schraudolph exp in bf16-bit space: exp(SCALE*x) ~= bitcast_bf16(
#   int16(round(x * SCALE*128*log2(e) + (127<<7) - 5.51)))
SCH_A = float(SCALE * 128.0 * np.log2(np.e))
SCH_B = float(127 * 128 - 5.51)

_NC_CACHE = {}


def _dram_bcast_ap(handle, parts):
    """[parts, len] AP reading a 1-D DRAM tensor broadcast across partitions."""
    ap = handle[:]
    return bass.AP(tensor=ap.tensor, offset=ap.offset, ap=[[0, parts]] + list(ap.ap))


def _schraud_units():
    # 24 exp units (qvp, c, j, qh, h) offloaded to DVE, spread across the
    # back 3/4 of the run. Offloading (qh1,h1) also lets the next block's
    # scores start while the DVE exp runs in parallel with ACT.
    units = set()
    for c in (1, 2, 3):
        for j in range(4):
            units.add((0, c, j, 1, 1))
    for c in (0, 1, 2, 3):
        for j in range(4):
            units.add((1, c, j, 1, 1))
    return units


def build_nc(zb_q=True, zb_k=True, zb_v=True, zb_o=True, g1=True, zbeta=True):
    nc = bacc.Bacc(None, target_bir_lowering=False)

    xt_d = nc.dram_tensor("xt", [D, NTOK], BF, kind="ExternalInput")
    xq_d = nc.dram_tensor("xq", [D, N * TH], BF, kind="ExternalInput")
    xres_d = nc.dram_tensor("xres", [N * TH, D], BF, kind="ExternalInput")
    wq_d = nc.dram_tensor("wq", [D, H * DK], BF, kind="ExternalInput")
    wk_d = nc.dram_tensor("wk", [D, H * DK], BF, kind="ExternalInput")
    wv_d = nc.dram_tensor("wv", [D, H * DV], BF, kind="ExternalInput")
    wo_d = nc.dram_tensor("wo", [H * DV, D], BF, kind="ExternalInput")
    bq_d = nc.dram_tensor("bq", [H * DK], F32, kind="ExternalInput")
    bk_d = nc.dram_tensor("bk", [H * DK], F32, kind="ExternalInput")
    bv_d = nc.dram_tensor("bv", [H * DV], F32, kind="ExternalInput")
    bo_d = nc.dram_tensor("bo", [D], F32, kind="ExternalInput")
    gamma_d = nc.dram_tensor("gamma", [D], F32, kind="ExternalInput")
    beta_d = nc.dram_tensor("beta", [D], F32, kind="ExternalInput")
    out_d = nc.dram_tensor("out", [N * TH, D], F32, kind="ExternalOutput")

    schraud = _schraud_units()

    with tile.TileContext(nc) as tc:
        with (
            tc.tile_pool(name="const", bufs=1) as constp,
            tc.tile_pool(name="xt", bufs=1) as xtp,
            tc.tile_pool(name="wts", bufs=1) as wtsp,
            tc.tile_pool(name="qkv", bufs=1) as qkvp,
            tc.tile_pool(name="attn", bufs=10) as attnp,
            tc.tile_pool(name="ctx", bufs=1) as ctxp,
            tc.tile_pool(name="tmpc", bufs=3) as tmpcp,
            tc.tile_pool(name="outs", bufs=1) as outsp,
            tc.tile_pool(name="fin", bufs=3) as finp,
            tc.tile_pool(name="ps_s", bufs=3, space="PSUM") as ps_s,
            tc.tile_pool(name="ps_d", bufs=1, space="PSUM") as ps_d,
            tc.tile_pool(name="ps_av", bufs=1, space="PSUM") as ps_av,
        ):
            # ---- constants
            ones_sb = constp.tile([128, 64], BF)
            nc.vector.memset(ones_sb, 1.0)
            eps_sb = constp.tile([128, 1], F32)
            nc.vector.memset(eps_sb, LN_EPS)
            dummy_sb = constp.tile([128, 1], F32)
            # warm the exp table set during the initial DMA wait
            nc.scalar.activation(dummy_sb, eps_sb, AF.Exp)
            warm_mv = constp.tile([128, 512], BF)
            nc.vector.memset(warm_mv, 0.0)

            if not (zb_q and zb_k):
                bq_sb = constp.tile([128, 4], F32)
                nc.sync.dma_start(out=bq_sb, in_=bq_d[:].rearrange("(c p) -> p c", p=128))
                bk_sb = constp.tile([128, 4], F32)
                nc.sync.dma_start(out=bk_sb, in_=bk_d[:].rearrange("(c p) -> p c", p=128))
            if not zb_v:
                bv_sb = constp.tile([128, H * DV], F32)
                nc.sync.dma_start(out=bv_sb, in_=_dram_bcast_ap(bv_d, 128))
            if not zb_o:
                bo_sb = constp.tile([128, D], F32)
                nc.sync.dma_start(out=bo_sb, in_=_dram_bcast_ap(bo_d, 128))
            if not g1:
                gamma_sb = constp.tile([128, D], F32)
                nc.sync.dma_start(out=gamma_sb, in_=_dram_bcast_ap(gamma_d, 128))
            if not zbeta:
                beta_sb = constp.tile([128, D], F32)
                nc.sync.dma_start(out=beta_sb, in_=_dram_bcast_ap(beta_d, 128))

            # ---- bulk loads: one batched DMA per tensor (fast startup),
            # issues only on sync/gpsimd (an issue costs the ACT engine
            # ~0.7us and ACT is a bottleneck). Ring order puts the
            # first-needed tensors (wq/xq-g0 and wk/xtv0) up front.
            def wload(handle, eng, nm):
                t_ = wtsp.tile([128, 4, 512], BF, tag=nm, name=nm)
                eng.dma_start(
                    out=t_, in_=handle[:].rearrange("(dj p) m -> p dj m", p=128)
                )
                return [t_[:, dj, :] for dj in range(4)]

            wq_sb = wload(wq_d, nc.gpsimd, "wq")
            xq_t = xtp.tile([128, 4, N * TH], BF, tag="xq", name="xqt")
            nc.sync.dma_start(
                out=xq_t[:, :, 0:512],
                in_=xq_d[:, 0:512].rearrange("(dj p) m -> p dj m", p=128),
            )
            xtv = [[None] * 4 for _ in range(4)]  # [dj][v]
            for dj in range(4):
                t_ = xtp.tile([128, 512], BF, tag=f"xt{dj}_0", name="xt0")
                nc.gpsimd.dma_start(out=t_, in_=xt_d[128 * dj : 128 * (dj + 1), 0:512])
                xtv[dj][0] = t_
            wk_sb = wload(wk_d, nc.sync, "wk")
            wv_sb = wload(wv_d, nc.gpsimd, "wv")
            nc.sync.dma_start(
                out=xq_t[:, :, 512:1024],
                in_=xq_d[:, 512:1024].rearrange("(dj p) m -> p dj m", p=128),
            )
            xq_sb = [xq_t[:, dj, :] for dj in range(4)]
            for v in range(1, 4):
                eng = (nc.gpsimd, nc.sync, nc.sync)[v - 1]
                for dj in range(4):
                    t_ = xtp.tile([128, 512], BF, tag=f"xt{dj}_{v}", name="xtv")
                    eng.dma_start(
                        out=t_,
                        in_=xt_d[128 * dj : 128 * (dj + 1), 512 * v : 512 * (v + 1)],
                    )
                    xtv[dj][v] = t_
            wo_sb = wload(wo_d, nc.gpsimd, "wo")
            xres_sb = []
            for g in range(2):
                t_ = outsp.tile([128, 4, 512], BF, tag=f"xres{g}", name="xrest")
                nc.gpsimd.dma_start(
                    out=t_,
                    in_=xres_d[512 * g : 512 * (g + 1), :].rearrange(
                        "(q p) d -> p q d", p=128
                    ),
                )
                xres_sb.extend(t_[:, q, :] for q in range(4))

            # ---- persistent tiles
            qt_sb = [qkvp.tile([128, N * TH], BF, tag=f"qt{j}", name="qt") for j in range(4)]
            kt_sb = [qkvp.tile([128, NTOK], BF, tag=f"kt{j}", name="kt") for j in range(4)]
            v_sb = [qkvp.tile([128, 512], BF, tag=f"v{m}", name="vm") for m in range(16)]
            ctx_sum = {}

            # proj/outproj accumulators ride the den/av PSUM banks as
            # transient ring tiles, alternating between the two.
            trans_state = [0]

            def trans_psum(name):
                pool, tag = ((ps_d, "d"), (ps_av, "av"))[trans_state[0] % 2]
                trans_state[0] += 1
                return pool.tile([128, 512], F32, tag=tag, name=name)

            filler = deque()

            def fill(n):
                for _ in range(min(n, len(filler))):
                    filler.popleft()()

            def drain_fill():
                while filler:
                    filler.popleft()()

            # ---- projection emitters
            def emit_qt(j, g):
                q_ps = trans_psum("qps")
                for dj in range(4):
                    nc.tensor.matmul(
                        q_ps,
                        wq_sb[dj][:, 128 * j : 128 * (j + 1)],
                        xq_sb[dj][:, 512 * g : 512 * (g + 1)],
                        start=(dj == 0),
                        stop=(dj == 3),
                    )
                dst = qt_sb[j][:, 512 * g : 512 * (g + 1)]
                if zb_q:
                    nc.vector.tensor_copy(dst, q_ps)
                else:
                    nc.vector.tensor_scalar_add(dst, q_ps, bq_sb[:, j : j + 1])

            def emit_kt(j, g):
                k_ps = trans_psum("kps")
                for dj in range(4):
                    nc.tensor.matmul(
                        k_ps,
                        wk_sb[dj][:, 128 * j : 128 * (j + 1)],
                        xtv[dj][g],
                        start=(dj == 0),
                        stop=(dj == 3),
                    )
                dst = kt_sb[j][:, 512 * g : 512 * (g + 1)]
                if zb_k:
                    nc.vector.tensor_copy(dst, k_ps)
                else:
                    nc.vector.tensor_scalar_add(dst, k_ps, bk_sb[:, j : j + 1])

            def emit_v(m):
                v_ps = trans_psum("vps")
                for dj in range(4):
                    nc.tensor.matmul(
                        v_ps,
                        xtv[dj][m // 4][:, 128 * (m % 4) : 128 * (m % 4 + 1)],
                        wv_sb[dj],
                        start=(dj == 0),
                        stop=(dj == 3),
                    )
                if zb_v:
                    nc.vector.tensor_copy(v_sb[m], v_ps)
                else:
                    nc.vector.tensor_tensor(v_sb[m], v_ps, bv_sb, OP.add)

            # ---- attention block, split so den/av trail the next block's
            # scores (keeps the exp-to-exp chain free of PE work)
            last_attn = [None]
            pending = deque()

            def scores_part(qvp, c, j, qh, a_h):
                qv = 2 * qvp + qh
                # Asymmetric rings: h0's single slot frees at exp(h0) of the
                # previous unit (just in time), h1's two slots free early --
                # both heads' score matmuls become ready together, so the PE
                # runs them as concurrent row-tile pairs.
                s_h = [
                    ps_s.tile([128, 1024], F32, tag="s0", name="s0", bufs=1),
                    ps_s.tile([128, 1024], F32, tag="s1", name="s1", bufs=2),
                ]
                for sc in range(4):
                    for h in range(2):
                        nc.tensor.matmul(
                            s_h[h][:, 256 * sc : 256 * (sc + 1)],
                            kt_sb[j][
                                64 * h : 64 * (h + 1),
                                512 * c + 128 * sc : 512 * c + 128 * (sc + 1),
                            ],
                            qt_sb[j][
                                64 * h : 64 * (h + 1),
                                256 * qv : 256 * (qv + 1),
                            ],
                            start=True,
                            stop=True,
                        )
                for h in range(2):
                    if (qvp, c, j, qh, h) in schraud:
                        dst = a_h[h][:, qh].bitcast(I16).rearrange("p s t -> p (s t)")
                        nc.vector.tensor_scalar(
                            dst, s_h[h], SCH_A, SCH_B, OP.mult, OP.add
                        )
                    else:
                        nc.scalar.activation(
                            a_h[h][:, qh], s_h[h], AF.Exp, scale=float(SCALE)
                        )

            def denav_part(qvp, c, j, a_h):
                d_ps = ps_d.tile([128, 512], F32, tag="d", name="d")
                for sc in range(4):
                    for h in range(2):
                        nc.tensor.matmul(
                            d_ps[64 * h : 64 * (h + 1), :],
                            ones_sb,
                            a_h[h][:, :, sc, :],
                            start=(sc == 0),
                            stop=(sc == 3),
                        )
                av_ps = ps_av.tile([128, 512], F32, tag="av", name="av")
                for sc in range(4):
                    for h in range(2):
                        nc.tensor.matmul(
                            av_ps[64 * h : 64 * (h + 1), :],
                            v_sb[4 * c + sc][
                                :, 64 * (2 * j + h) : 64 * (2 * j + h + 1)
                            ],
                            a_h[h][:, :, sc, :],
                            start=(sc == 0),
                            stop=(sc == 3),
                        )
                rb = attnp.tile([128, 512], F32, tag="rb", name="rb")
                nc.vector.reciprocal_approx_fast(rb, d_ps)
                key = (qvp, j)
                if key not in ctx_sum:
                    cs = ctxp.tile([128, 512], BF, tag=f"cs{qvp}_{j}", name="cs")
                    ctx_sum[key] = cs
                    nc.vector.tensor_tensor(cs, av_ps, rb, OP.mult)
                else:
                    tmp = tmpcp.tile([128, 512], BF, tag="tmp", name="tmp")
                    nc.vector.tensor_tensor(tmp, av_ps, rb, OP.mult)
                    nc.vector.tensor_tensor(ctx_sum[key], ctx_sum[key], tmp, OP.add)
                last_attn[0] = a_h[1]

            def flush_all_pending():
                while pending:
                    denav_part(*pending.popleft())

            def emit_block(qvp, c, j):
                a_h = [
                    attnp.tile([128, 2, 4, 256], BF, tag="attn", name="a0"),
                    attnp.tile([128, 2, 4, 256], BF, tag="attn", name="a1"),
                ]
                scores_part(qvp, c, j, 0, a_h)
                # denav runs two blocks behind its scores: enough slack to
                # absorb projection bursts without stalling the exp stream
                if len(pending) >= 2:
                    denav_part(*pending.popleft())
                fill(1)
                scores_part(qvp, c, j, 1, a_h)
                fill(1)
                pending.append((qvp, c, j, a_h))

            # ---- out-projection + LayerNorm stats (normalize deferred)
            res_tiles, mv_tiles = [None] * 8, [None] * 8

            def emit_outproj_group(qvp, qh, tch):
                qv = 2 * qvp + qh
                o_ps = trans_psum("o")
                for j in range(4):
                    nc.tensor.matmul(
                        o_ps,
                        ctx_sum[(qvp, j)][
                            :, 256 * qh + 128 * tch : 256 * qh + 128 * (tch + 1)
                        ],
                        wo_sb[j],
                        start=(j == 0),
                        stop=(j == 3),
                    )
                r = 2 * qv + tch
                res = outsp.tile([128, D], F32, tag=f"res{r}", name="res")
                nc.vector.scalar_tensor_tensor(
                    res, o_ps, 1.0 / N, xres_sb[r], OP.mult, OP.add
                )
                if not zb_o:
                    nc.vector.tensor_tensor(res, res, bo_sb, OP.add)
                stats = finp.tile([128, 6], F32, tag="stats", name="st")
                nc.vector.bn_stats(stats, res)
                mv = outsp.tile([128, 2], F32, tag=f"mv{r}", name="mv")
                nc.vector.bn_aggr(mv, stats)
                res_tiles[r], mv_tiles[r] = res, mv

            # ================= emission schedule =================
            # PE warm-up: ~5us of junk matmuls on constants so the HAM
            # clock gate is at 8/8 before the first real projection (the
            # first ~3.4us of PE work otherwise runs at half clock).
            for _ in range(24):
                w_ps = trans_psum("warm")
                nc.tensor.matmul(
                    w_ps[0:64, :], ones_sb, warm_mv, start=True, stop=True
                )

            # qvp0: kt chunks are emitted one block ahead of use so phase
            # boundaries don't stall the exp stream; V chunks for var c
            # land before the first denav of the phase (which fires inside
            # the phase's second block).
            kt_seq = [(j, c) for c in range(4) for j in range(4)]
            emit_kt(*kt_seq[0])
            for c in range(4):
                for j in range(4):
                    if c == 0:
                        emit_qt(j, 0)
                    nxt = 4 * c + j + 1
                    if nxt < 16:
                        emit_kt(*kt_seq[nxt])
                    emit_block(0, c, j)
                    if c == 0 and j < 2:
                        emit_v(2 * j)
                        emit_v(2 * j + 1)
                    if c < 3:
                        # one V chunk of the NEXT phase per block, so no
                        # 4-chunk projection burst stalls the exp stream
                        # at the phase boundary
                        emit_v(4 * (c + 1) + j)
                    else:
                        # spread the qvp1 q-projections over phase 3
                        emit_qt(j, 1)

            for c in range(4):
                for j in range(4):
                    emit_block(1, c, j)
                    if c == 0 and j == 0:
                        # ctx_sum[(0,*)] completed by the denav flushed in
                        # this block; qvp0 out-projections become filler.
                        for qh in range(2):
                            for tch in range(2):
                                filler.append(
                                    lambda qh=qh, tch=tch: emit_outproj_group(
                                        0, qh, tch
                                    )
                                )
            flush_all_pending()
            drain_fill()
            for qh in range(2):
                for tch in range(2):
                    emit_outproj_group(1, qh, tch)

            # ---- deferred LayerNorm normalize, gated behind the last
            # attention exp so the exp->sqrt table switch happens once.
            eps_gate = constp.tile([128, 1], F32)
            nc.vector.tensor_scalar(
                eps_gate, last_attn[0][:, 0, 0, 0:1], 0.0, LN_EPS, OP.mult, OP.add
            )
            for r in range(8):
                rstd = finp.tile([128, 1], F32, tag="rstd", name="rst")
                nc.scalar.activation(rstd, mv_tiles[r][:, 1:2], AF.Sqrt, bias=eps_gate)
                rstd2 = finp.tile([128, 1], F32, tag="rstd2", name="rs2")
                nc.vector.reciprocal(rstd2, rstd)
                y = finp.tile([128, D], F32, tag="y", name="y")
                nc.vector.tensor_scalar(
                    y, res_tiles[r], mv_tiles[r][:, 0:1], rstd2, OP.subtract, OP.mult
                )
                if not g1:
                    nc.vector.tensor_tensor(y, y, gamma_sb, OP.mult)
                if not zbeta:
                    nc.vector.tensor_tensor(y, y, beta_sb, OP.add)
                eng = (nc.sync, nc.gpsimd)[r % 2]
                eng.dma_start(out=out_d[128 * r : 128 * (r + 1), :], in_=y)

    nc.compile()
    return nc


def get_nc(spec=(True,) * 6):
    if spec not in _NC_CACHE:
        _NC_CACHE[spec] = build_nc(*spec)
    return _NC_CACHE[spec]


def input_spec(bq, bk, bv, bo, gamma, beta):
    return (
        not np.any(bq),
        not np.any(bk),
        not np.any(bv),
        not np.any(bo),
        bool(np.all(np.asarray(gamma, np.float32) == 1.0)),
        not np.any(beta),
    )


def make_in_maps(x, Wq, bq, Wk, bk, Wv, bv, Wo, bo, gamma, beta):
    bf = ml_dtypes.bfloat16
    x = np.asarray(x, np.float32)
    wq16 = np.ascontiguousarray(np.asarray(Wq, np.float32).astype(bf))
    wk16 = np.ascontiguousarray(np.asarray(Wk, np.float32).astype(bf))
    wv16 = np.ascontiguousarray(np.asarray(Wv, np.float32).astype(bf))
    wo16 = np.ascontiguousarray(np.asarray(Wo, np.float32).astype(bf))
    vecs = {
        "bq": np.ascontiguousarray(np.asarray(bq, np.float32)),
        "bk": np.ascontiguousarray(np.asarray(bk, np.float32)),
        "bv": np.ascontiguousarray(np.asarray(bv, np.float32)),
        "bo": np.ascontiguousarray(np.asarray(bo, np.float32)),
        "gamma": np.ascontiguousarray(np.asarray(gamma, np.float32)),
        "beta": np.ascontiguousarray(np.asarray(beta, np.float32)),
    }
    in_maps = []
    for ci in range(8):
        b, th = ci // 2, ci % 2
        xb = x[:, b]  # [N, T, D]
        xt = np.ascontiguousarray(
            xb.transpose(2, 0, 1).reshape(D, NTOK)
        ).astype(bf)
        xq = np.ascontiguousarray(
            xb[:, th * TH : (th + 1) * TH, :].transpose(2, 0, 1).reshape(D, N * TH)
        ).astype(bf)
        xres = np.ascontiguousarray(
            xb[:, th * TH : (th + 1) * TH, :].reshape(N * TH, D)
        ).astype(bf)
        m = {
            "xt": xt,
            "xq": xq,
            "xres": xres,
            "wq": wq16,
            "wk": wk16,
            "wv": wv16,
            "wo": wo16,
        }
        m.update(vecs)
        in_maps.append(m)
    return in_maps


def assemble(results):
    out = np.empty((N, B, T, D), np.float32)
    for ci in range(8):
        b, th = ci // 2, ci % 2
        o = np.asarray(results[ci]["out"], np.float32).reshape(N, TH, D)
        out[:, b, th * TH : (th + 1) * TH, :] = o
    return out


def kernel(**inputs) -> np.ndarray:
    spec = input_spec(
        inputs["bq"], inputs["bk"], inputs["bv"],
        inputs["bo"], inputs["gamma"], inputs["beta"],
    )
    nc = get_nc(spec)
    in_maps = make_in_maps(**inputs)
    res = run_bass_kernel_spmd(nc, in_maps, core_ids=list(range(8)), trace=False)
    return assemble(res.results)
